# revision 1
# baseline (speedup 1.0000x reference)
"""Trainium2 Bass kernel: multi-head attention (B=2, S=2048, E=1024, H=16).

Sharding: 8 cores = 2 batches x 4 head-groups. Core c handles batch c//4 and
heads [4*(c%4), 4*(c%4)+4) (256 feature columns of the projections).

Per-core device program (all matmuls in fp32r):
  - inputs: xT [E,S] (host-transposed x[b]), wqT/wkT/wvT [E,256] (host-
    transposed row-slices of Wq/Wk/Wv), woT [256,E] (host-transposed column
    slice of Wo).
  - qT,kT [256,S] = (x @ W^T)^T per head-group, computed directly in [f,s]
    layout; v [S,256] in [s,f] layout with a ones column appended per head.
  - per (head, qi-chunk): scores^T tiles [128 kj, 512 qi] on PE, exp on ACT
    (sm_scale folded into the activation scale), attn@v accumulated on PE with
    the ones column producing the softmax denominator in partition 64,
    then reciprocal + GPSIMD partition-broadcast + multiply to normalize;
    output kept in [f, s] layout for the output projection.
  - out_partial [S,E] = o^T^T @ Wo^T column-slice; host sums 4 partials per
    batch and adds bo.
"""

import numpy as np

import concourse.tile as tile
import concourse.mybir as mybir
from concourse import bacc
from concourse.bass_utils import run_bass_kernel_spmd

B, S, E, H, D = 2, 2048, 1024, 16, 64
NCORES = 8
GPB = NCORES // B      # head-groups (cores) per batch = 4
HPC = H // GPB         # heads per core = 4
FPC = HPC * D          # feature cols per core = 256
SM = float(D) ** -0.5  # softmax scale

F32 = mybir.dt.float32
F32R = mybir.dt.float32r

P = 128
NE = E // P            # 8 e-tiles
NST = S // P           # 16 s-tiles (key tiles)
NQ = 4                 # qi chunks
QC = S // NQ           # 512
KTG = 2                # k-tiles per psum/exp group
NKG = NST // KTG       # 8 groups
FT = FPC // P          # 2 f-tiles per core


def _round_fp32r(a: np.ndarray) -> np.ndarray:
    """Round fp32 to the fp32r encoding (RNE to 12-bit mantissa)."""
    u = np.ascontiguousarray(a, dtype=np.float32).view(np.uint32)
    lo = u & np.uint32(0xFFF)
    base = u & ~np.uint32(0xFFF)
    rup = (lo > 0x800) | ((lo == 0x800) & (((base >> np.uint32(12)) & np.uint32(1)) == 1))
    out = base + (rup.astype(np.uint32) << np.uint32(12))
    return out.view(np.float32)


def _build():
    nc = bacc.Bacc("TRN2", target_bir_lowering=False, debug=False)

    xT_d = nc.dram_tensor("xT", [E, S], F32R, kind="ExternalInput")
    wq_d = nc.dram_tensor("wqT", [E, FPC], F32R, kind="ExternalInput")
    wk_d = nc.dram_tensor("wkT", [E, FPC], F32R, kind="ExternalInput")
    wv_d = nc.dram_tensor("wvT", [E, FPC], F32R, kind="ExternalInput")
    wo_d = nc.dram_tensor("woT", [FPC, E], F32R, kind="ExternalInput")
    ones_lhs_d = nc.dram_tensor("ones_lhs", [1, D], F32R, kind="ExternalInput")
    ones_col_d = nc.dram_tensor("ones_col", [P, HPC, 1], F32R, kind="ExternalInput")
    out_d = nc.dram_tensor("out", [S, E], F32, kind="ExternalOutput")

    with tile.TileContext(nc) as tc:
        with (
            tc.tile_pool(name="wpool", bufs=1) as wpool,
            tc.tile_pool(name="xpool", bufs=1) as xpool,
            tc.tile_pool(name="qkpool", bufs=1) as qkpool,
            tc.tile_pool(name="vpool", bufs=1) as vpool,
            tc.tile_pool(name="opool", bufs=1) as opool,
            tc.tile_pool(name="epool", bufs=3) as epool,
            tc.tile_pool(name="spool", bufs=2) as spool,
            tc.tile_pool(name="outpool", bufs=3) as outpool,
            tc.tile_pool(name="pspool", bufs=2, space="PSUM") as pspool,
            tc.tile_pool(name="popool", bufs=2, space="PSUM") as popool,
            tc.tile_pool(name="oaccpool", bufs=2, space="PSUM") as oaccpool,
        ):
            # ---- weights / constants -------------------------------------
            wq = wpool.tile([P, NE, FPC], F32R, name="wq")
            wk = wpool.tile([P, NE, FPC], F32R, name="wk")
            wv = wpool.tile([P, NE, FPC], F32R, name="wv")
            wo = wpool.tile([P, FT, E], F32R, name="wo")
            ones = wpool.tile([1, D], F32R, name="ones")
            wk_r = wk_d.ap().rearrange("(t p) f -> p t f", p=P)
            wq_r = wq_d.ap().rearrange("(t p) f -> p t f", p=P)
            # f-tile-0 halves first: only they gate the first score matmuls;
            # the ft1 halves ride behind the early x chunks.
            nc.sync.dma_start(out=wk[:, :, 0:P], in_=wk_r[:, :, 0:P])
            nc.sync.dma_start(out=wq[:, :, 0:P], in_=wq_r[:, :, 0:P])

            # ---- x^T (chunk-major DMA so compute starts early) -----------
            xT_r = xT_d.ap().rearrange("(t p) s -> p t s", p=P)
            xts = [
                xpool.tile([P, S], F32R, name=f"xt{et}", tag=f"xt{et}")
                for et in range(NE)
            ]
            for cq in range(NQ):
                csl = slice(cq * QC, (cq + 1) * QC)
                for et in range(NE):
                    nc.sync.dma_start(out=xts[et][:, csl], in_=xT_r[:, et, csl])
                if cq == 0:
                    nc.sync.dma_start(
                        out=wv, in_=wv_d.ap().rearrange("(t p) f -> p t f", p=P)
                    )
                    nc.sync.dma_start(out=ones, in_=ones_lhs_d.ap())
                elif cq == 2:
                    nc.sync.dma_start(out=wk[:, :, P:FPC], in_=wk_r[:, :, P:FPC])
                    nc.sync.dma_start(out=wq[:, :, P:FPC], in_=wq_r[:, :, P:FPC])

            nc.sync.dma_start(out=wo, in_=wo_d.ap().rearrange("(t p) g -> p t g", p=P))

            # ---- v projection: v[s, f] with ones col per head ------------
            v_tiles = [
                vpool.tile([P, HPC, D + 1], F32R, name=f"v{st}", tag=f"v{st}")
                for st in range(NST)
            ]

            def proj_v(st):
                vt = v_tiles[st]
                nc.sync.dma_start(out=vt[:, :, D : D + 1], in_=ones_col_d.ap())
                ps_v = popool.tile([P, FPC], F32, name="ps_v", tag="po")
                for et in range(NE):
                    nc.tensor.matmul(
                        ps_v,
                        xts[et][:, st * P : (st + 1) * P],
                        wv[:, et, :],
                        start=(et == 0),
                        stop=(et == NE - 1),
                    )
                nc.vector.tensor_copy(
                    vt[:, :, 0:D], ps_v.rearrange("p (h d) -> p h d", d=D)
                )

            # ---- q^T / k^T projections: [f, s] ---------------------------
            def proj_T(w_tile, dst_tiles, which, ft, cq):
                ps = popool.tile([P, QC], F32, name=f"ps_{which}", tag="po")
                for et in range(NE):
                    nc.tensor.matmul(
                        ps,
                        w_tile[:, et, ft * P : (ft + 1) * P],
                        xts[et][:, cq * QC : (cq + 1) * QC],
                        start=(et == 0),
                        stop=(et == NE - 1),
                    )
                nc.vector.tensor_copy(
                    dst_tiles[ft][:, cq * QC : (cq + 1) * QC], ps
                )

            kts = [qkpool.tile([P, S], F32R, name=f"kt{ft}", tag=f"kt{ft}") for ft in range(FT)]
            qts = [qkpool.tile([P, S], F32R, name=f"qt{ft}", tag=f"qt{ft}") for ft in range(FT)]
            ots = [opool.tile([P, S], F32R, name=f"ot{ft}", tag=f"ot{ft}") for ft in range(FT)]

            # Filler machinery: generators that emit one PE-side instruction
            # per next() call. attn_core drains a couple of units after each
            # kt step, so independent matmul work lands inside the PE idle
            # gaps of the ACT-bound attention inner loop instead of between
            # cores (the PE executes its stream in order).
            from collections import deque

            fillers = deque()

            def pump(n):
                for _ in range(n):
                    while fillers:
                        try:
                            next(fillers[0])
                            break
                        except StopIteration:
                            fillers.popleft()
                    else:
                        return

            def attn_core(pair, cq, per_kt=2):
                """Heads 2*pair, 2*pair+1 for query chunk cq; the two heads'
                score matmuls run concurrently on PE row-groups 0-63/64-127.
                Returns the two accumulation psum tiles (rows 0..63 =
                sum(exp*v), row 64 = sum(exp))."""
                ft = pair
                csl = slice(cq * QC, (cq + 1) * QC)
                ps_o = [
                    oaccpool.tile([D + 1, QC], F32, name=f"ps_o{s}", tag="oacc")
                    for s in range(2)
                ]
                for kt in range(NST):
                    et_t = epool.tile([P, 2, QC], F32R, name="et_t", tag="et_t")
                    ps_s = pspool.tile([P, 2, QC], F32, name="ps_s", tag="ps_s")
                    for sub in range(2):
                        lo, hi = sub * D, (sub + 1) * D
                        nc.tensor.matmul(
                            ps_s[:, sub, :],
                            kts[ft][lo:hi, kt * P : (kt + 1) * P],
                            qts[ft][lo:hi, csl],
                            start=True,
                            stop=True,
                        )
                    nc.scalar.activation(
                        out=et_t,
                        in_=ps_s,
                        func=mybir.ActivationFunctionType.Exp,
                        scale=SM,
                    )
                    for sub in range(2):
                        nc.tensor.matmul(
                            ps_o[sub],
                            v_tiles[kt][:, 2 * pair + sub, :],
                            et_t[:, sub, :],
                            start=(kt == 0),
                            stop=(kt == NST - 1),
                        )
                    if kt > 0:
                        pump(per_kt)
                return ps_o

            def attn_drain(ps_o):
                """Copy both accumulators (incl. the sum row) to SBUF right
                away so the psum slots free early."""
                o_full = []
                for sub in range(2):
                    of = epool.tile([D + 1, QC], F32, name="o_hat", tag="o_hat", bufs=4)
                    nc.vector.tensor_copy(of, ps_o[sub])
                    o_full.append(of)
                return o_full

            def bcast_recip(o_full):
                """Reciprocal of each sum row, partition-broadcast on the
                (otherwise idle) GPSIMD engine. No PE/ACT work."""
                bcs = []
                for sub in range(2):
                    rec = spool.tile([1, QC], F32, name="rec", tag="rec", bufs=1)
                    nc.vector.reciprocal(rec, o_full[sub][D : D + 1, :])
                    bc = spool.tile([D, QC], F32, name="bc", tag="bc", bufs=4)
                    nc.gpsimd.partition_broadcast(bc, rec)
                    bcs.append(bc)
                return bcs

            def attn_finish(pair, cq, o_full):
                """Normalize a pair-0 chunk (full-width multiply)."""
                csl = slice(cq * QC, (cq + 1) * QC)
                bcs = bcast_recip(o_full)
                for sub in range(2):
                    lo, hi = sub * D, (sub + 1) * D
                    nc.vector.tensor_mul(
                        ots[pair][lo:hi, csl], o_full[sub][0:D, :], bcs[sub]
                    )

            def finish_outproj_units(cq, o_full, bcs, tail=False):
                """Pair-1 normalize pipelined with the output projection at
                s-tile granularity (shortens the kernel tail). In the tail
                the PSUM->SBUF copies ride the idle ACT engine instead of
                DVE."""
                for sti in range(NQ):
                    st = cq * NQ + sti
                    ssl = slice(sti * P, (sti + 1) * P)
                    for sub in range(2):
                        lo, hi = sub * D, (sub + 1) * D
                        nc.vector.tensor_mul(
                            ots[1][lo:hi, st * P : (st + 1) * P],
                            o_full[sub][0:D, ssl],
                            bcs[sub][:, ssl],
                        )
                    yield
                    out_sb = outpool.tile([P, E], F32, name="out_sb", tag="out_sb")
                    for gc in range(2):
                        ps_out = popool.tile([P, QC], F32, name="ps_out", tag="po")
                        for ft in range(FT):
                            nc.tensor.matmul(
                                ps_out,
                                ots[ft][:, st * P : (st + 1) * P],
                                wo[:, ft, gc * QC : (gc + 1) * QC],
                                start=(ft == 0),
                                stop=(ft == FT - 1),
                            )
                            yield
                        if tail:
                            nc.scalar.activation(
                                out=out_sb[:, gc * QC : (gc + 1) * QC],
                                in_=ps_out,
                                func=mybir.ActivationFunctionType.Copy,
                            )
                        else:
                            nc.vector.tensor_copy(
                                out_sb[:, gc * QC : (gc + 1) * QC], ps_out
                            )
                        yield
                    nc.sync.dma_start(
                        out=out_d.ap()[st * P : (st + 1) * P, :], in_=out_sb
                    )

            # Emission order = scheduler priority. Attention cores are
            # emitted right after the projections of their own chunk, so the
            # first exp fires as soon as chunk-0 data exists; later-chunk
            # projections backfill PE whenever attention is dep-blocked.
            def proj1_units():
                for cq in range(NQ):
                    for w_tile, dst, which in ((wk, kts, "k1"), (wq, qts, "q1")):
                        ps = popool.tile([P, QC], F32, name=f"ps_{which}", tag="po")
                        for et in range(NE):
                            nc.tensor.matmul(
                                ps,
                                w_tile[:, et, P : 2 * P],
                                xts[et][:, cq * QC : (cq + 1) * QC],
                                start=(et == 0),
                                stop=(et == NE - 1),
                            )
                            yield
                        nc.vector.tensor_copy(
                            dst[1][:, cq * QC : (cq + 1) * QC], ps
                        )
                        yield

            for cq in range(NQ):
                proj_T(wk, kts, "k0", 0, cq)
                proj_T(wq, qts, "q0", 0, cq)
                for st in range(cq * NQ, (cq + 1) * NQ):
                    proj_v(st)

            PER_KT = {(0, 1): 2}
            for pair in range(2):
                for cq in range(NQ):
                    ps_o = attn_core(pair, cq, per_kt=PER_KT.get((pair, cq), 2 if pair else 1))
                    of = attn_drain(ps_o)
                    if pair == 0:
                        attn_finish(pair, cq, of)
                    elif cq < NQ - 1:
                        bcs = bcast_recip(of)
                        fillers.append(finish_outproj_units(cq, of, bcs))
                    else:
                        # tail chunk: broadcast via a PE matmul (shortest
                        # latency chain right after the last core)
                        bcs = []
                        for sub in range(2):
                            rec = spool.tile([1, QC], F32, name="rec", tag="rec", bufs=1)
                            nc.vector.reciprocal(rec, of[sub][D : D + 1, :])
                            rec_r = spool.tile([1, QC], F32R, name="rec_r", tag="rec_r", bufs=1)
                            nc.vector.tensor_copy(rec_r, rec)
                            ps_bc = popool.tile([D, QC], F32, name="ps_bc", tag="po")
                            nc.tensor.matmul(ps_bc, ones, rec_r, start=True, stop=True)
                            bcs.append(ps_bc)
                        fillers.appendleft(
                            finish_outproj_units(cq, of, bcs, tail=True)
                        )
                    if pair == 0 and cq == 0:
                        fillers.append(proj1_units())
            # drain remaining fillers (the last chunk's output projection)
            while fillers:
                pump(64)

    nc.compile()
    return nc


_NC_CACHE = None


def _get_nc():
    global _NC_CACHE
    if _NC_CACHE is None:
        _NC_CACHE = _build()
    return _NC_CACHE


def make_in_maps(x, Wq, Wk, Wv, Wo):
    in_maps = []
    xTs = [_round_fp32r(x[b].T) for b in range(B)]
    for c in range(NCORES):
        b, hg = c // GPB, c % GPB
        fsl = slice(hg * FPC, (hg + 1) * FPC)
        in_maps.append({
            "xT": xTs[b],
            "wqT": _round_fp32r(Wq[fsl, :].T),
            "wkT": _round_fp32r(Wk[fsl, :].T),
            "wvT": _round_fp32r(Wv[fsl, :].T),
            "woT": _round_fp32r(Wo[:, fsl].T),
            "ones_lhs": np.ones((1, D), dtype=np.float32),
            "ones_col": np.ones((P, HPC, 1), dtype=np.float32),
        })
    return in_maps


def kernel(x, Wq, bq, Wk, bk, Wv, bv, Wo, bo):
    x = np.asarray(x, dtype=np.float32)
    Wq, Wk, Wv, Wo = (np.asarray(a, dtype=np.float32) for a in (Wq, Wk, Wv, Wo))
    bq, bk, bv, bo = (np.asarray(a, dtype=np.float32) for a in (bq, bk, bv, bo))
    if np.any(bq) or np.any(bk) or np.any(bv):
        # fall back: fold nonzero projection biases into an augmented input
        # row is not implemented; biases are zero for this problem spec.
        raise NotImplementedError("nonzero projection biases not supported")

    nc = _get_nc()
    in_maps = make_in_maps(x, Wq, Wk, Wv, Wo)
    res = run_bass_kernel_spmd(nc, in_maps, core_ids=list(range(NCORES)))
    out = np.empty((B, S, E), dtype=np.float32)
    for b in range(B):
        acc = res.results[b * GPB]["out"].astype(np.float32).copy()
        for hg in range(1, GPB):
            acc += res.results[b * GPB + hg]["out"]
        out[b] = acc
    out += bo[None, None, :]
    return out



# revision 21
# speedup vs baseline: 1.0220x; 1.0220x over previous
"""Trainium2 Bass kernel: multi-head attention (B=2, S=2048, E=1024, H=16).

Sharding: 8 cores = 2 batches x 4 head-groups. Core c handles batch c//4 and
heads [4*(c%4), 4*(c%4)+4) (256 feature columns of the projections).

Per-core device program (all matmuls in fp32r):
  - inputs: xT [E,S] (host-transposed x[b]), wqT/wkT/wvT [E,256] (host-
    transposed row-slices of Wq/Wk/Wv), woT [256,E] (host-transposed column
    slice of Wo).
  - qT,kT [256,S] = (x @ W^T)^T per head-group, computed directly in [f,s]
    layout; v [S,256] in [s,f] layout with a ones column appended per head.
  - per (head, qi-chunk): scores^T tiles [128 kj, 512 qi] on PE, exp on ACT
    (sm_scale folded into the activation scale), attn@v accumulated on PE with
    the ones column producing the softmax denominator in partition 64,
    then reciprocal + GPSIMD partition-broadcast + multiply to normalize;
    output kept in [f, s] layout for the output projection.
  - out_partial [S,E] = o^T^T @ Wo^T column-slice; host sums 4 partials per
    batch and adds bo.
"""

import numpy as np

import concourse.tile as tile
import concourse.mybir as mybir
from concourse import bacc
from concourse.bass_utils import run_bass_kernel_spmd

B, S, E, H, D = 2, 2048, 1024, 16, 64
NCORES = 8
GPB = NCORES // B      # head-groups (cores) per batch = 4
HPC = H // GPB         # heads per core = 4
FPC = HPC * D          # feature cols per core = 256
SM = float(D) ** -0.5  # softmax scale

F32 = mybir.dt.float32
F32R = mybir.dt.float32r
BF16 = mybir.dt.bfloat16

P = 128
NE = E // P            # 8 e-tiles
NST = S // P           # 16 s-tiles (key tiles)
NQ = 4                 # qi chunks
QC = S // NQ           # 512
KTG = 2                # k-tiles per psum/exp group
NKG = NST // KTG       # 8 groups
FT = FPC // P          # 2 f-tiles per core


def _round_fp32r(a: np.ndarray) -> np.ndarray:
    """Round fp32 to the fp32r encoding (RNE to 12-bit mantissa)."""
    u = np.ascontiguousarray(a, dtype=np.float32).view(np.uint32)
    lo = u & np.uint32(0xFFF)
    base = u & ~np.uint32(0xFFF)
    rup = (lo > 0x800) | ((lo == 0x800) & (((base >> np.uint32(12)) & np.uint32(1)) == 1))
    out = base + (rup.astype(np.uint32) << np.uint32(12))
    return out.view(np.float32)


def _build():
    nc = bacc.Bacc("TRN2", target_bir_lowering=False, debug=False)

    xT_d = nc.dram_tensor("xT", [E, S], BF16, kind="ExternalInput")
    wq_d = nc.dram_tensor("wqT", [E, FPC], BF16, kind="ExternalInput")
    wk_d = nc.dram_tensor("wkT", [E, FPC], BF16, kind="ExternalInput")
    wv_d = nc.dram_tensor("wvT", [E, FPC], BF16, kind="ExternalInput")
    wo_d = nc.dram_tensor("woT", [FPC, E], F32R, kind="ExternalInput")
    ones_lhs_d = nc.dram_tensor("ones_lhs", [1, D], F32R, kind="ExternalInput")
    ones_col_d = nc.dram_tensor("ones_col", [P, HPC, 1], F32R, kind="ExternalInput")
    out_d = nc.dram_tensor("out", [S, E], F32, kind="ExternalOutput")

    with tile.TileContext(nc) as tc:
        with (
            tc.tile_pool(name="wpool", bufs=1) as wpool,
            tc.tile_pool(name="xpool", bufs=1) as xpool,
            tc.tile_pool(name="qkpool", bufs=1) as qkpool,
            tc.tile_pool(name="vpool", bufs=1) as vpool,
            tc.tile_pool(name="opool", bufs=1) as opool,
            tc.tile_pool(name="epool", bufs=3) as epool,
            tc.tile_pool(name="spool", bufs=2) as spool,
            tc.tile_pool(name="outpool", bufs=3) as outpool,
            tc.tile_pool(name="pspool", bufs=2, space="PSUM") as pspool,
            tc.tile_pool(name="popool", bufs=2, space="PSUM") as popool,
            tc.tile_pool(name="oaccpool", bufs=2, space="PSUM") as oaccpool,
        ):
            # ---- weights / constants -------------------------------------
            wq = wpool.tile([P, NE, FPC], BF16, name="wq")
            wk = wpool.tile([P, NE, FPC], BF16, name="wk")
            wv = wpool.tile([P, NE, FPC], BF16, name="wv")
            wo = wpool.tile([P, FT, E], F32R, name="wo")
            ones = wpool.tile([1, D], F32R, name="ones")

            wk_r = wk_d.ap().rearrange("(t p) f -> p t f", p=P)
            wq_r = wq_d.ap().rearrange("(t p) f -> p t f", p=P)
            wv_r = wv_d.ap().rearrange("(t p) f -> p t f", p=P)

            # ---- x^T (chunk-major DMA so compute starts early) -----------
            xT_r = xT_d.ap().rearrange("(t p) s -> p t s", p=P)
            # x in bf16: halves the startup-critical HBM traffic; the ~0.2%
            # relative quantization is well inside the 2e-2 tolerance.
            xts = [
                xpool.tile([P, S], BF16, name=f"xt{et}", tag=f"xt{et}")
                for et in range(NE)
            ]
            # Startup: each dma_start costs ~625ns serialized HWDGE + 650ns
            # DGE + 900ns sem-prop, so keep DMA count low but split off just
            # the first matmul's deps (wk et0 slice + x et0 chunk0) so the PE
            # starts ~3µs earlier. The k0 accumulation then rides the x DMA
            # stream et by et.
            nc.sync.dma_start(out=wk[:, 0:1, 0:P], in_=wk_r[:, 0:1, 0:P])
            nc.sync.dma_start(out=xts[0][:, 0:QC], in_=xT_r[:, 0, 0:QC])
            nc.sync.dma_start(out=wk[:, 1:NE, 0:P], in_=wk_r[:, 1:NE, 0:P])
            for et in range(1, NE):
                nc.sync.dma_start(out=xts[et][:, 0:QC], in_=xT_r[:, et, 0:QC])
            nc.sync.dma_start(out=wq[:, :, 0:P], in_=wq_r[:, :, 0:P])
            nc.sync.dma_start(out=wv[:, 0:4, :], in_=wv_r[:, 0:4, :])
            nc.sync.dma_start(out=wv[:, 4:NE, :], in_=wv_r[:, 4:NE, :])
            nc.sync.dma_start(out=ones, in_=ones_lhs_d.ap())
            for cq in range(1, NQ):
                csl = slice(cq * QC, (cq + 1) * QC)
                for et in range(NE):
                    nc.sync.dma_start(out=xts[et][:, csl], in_=xT_r[:, et, csl])
                if cq == 2:
                    nc.sync.dma_start(out=wk[:, :, P:FPC], in_=wk_r[:, :, P:FPC])
                    nc.sync.dma_start(out=wq[:, :, P:FPC], in_=wq_r[:, :, P:FPC])

            nc.sync.dma_start(out=wo, in_=wo_d.ap().rearrange("(t p) g -> p t g", p=P))

            # ---- v projection: v[s, f] with ones col per head ------------
            v_tiles = [
                vpool.tile([P, HPC, D + 1], F32R, name=f"v{st}", tag=f"v{st}")
                for st in range(NST)
            ]

            def proj_v(st):
                vt = v_tiles[st]
                nc.sync.dma_start(out=vt[:, :, D : D + 1], in_=ones_col_d.ap())
                ps_v = popool.tile([P, FPC], F32, name="ps_v", tag="po")
                for et in range(NE):
                    nc.tensor.matmul(
                        ps_v,
                        xts[et][:, st * P : (st + 1) * P],
                        wv[:, et, :],
                        start=(et == 0),
                        stop=(et == NE - 1),
                    )
                nc.vector.tensor_copy(
                    vt[:, :, 0:D], ps_v.rearrange("p (h d) -> p h d", d=D)
                )

            # ---- q^T / k^T projections: [f, s] ---------------------------
            def proj_T(w_tile, dst_tiles, which, ft, cq):
                ps = popool.tile([P, QC], F32, name=f"ps_{which}", tag="po")
                for et in range(NE):
                    nc.tensor.matmul(
                        ps,
                        w_tile[:, et, ft * P : (ft + 1) * P],
                        xts[et][:, cq * QC : (cq + 1) * QC],
                        start=(et == 0),
                        stop=(et == NE - 1),
                    )
                nc.vector.tensor_copy(
                    dst_tiles[ft][:, cq * QC : (cq + 1) * QC], ps
                )

            kts = [qkpool.tile([P, S], F32R, name=f"kt{ft}", tag=f"kt{ft}") for ft in range(FT)]
            qts = [qkpool.tile([P, S], F32R, name=f"qt{ft}", tag=f"qt{ft}") for ft in range(FT)]
            ots = [opool.tile([P, S], F32R, name=f"ot{ft}", tag=f"ot{ft}") for ft in range(FT)]

            # Filler machinery: generators that emit one PE-side instruction
            # per next() call. attn_core drains a couple of units after each
            # kt step, so independent matmul work lands inside the PE idle
            # gaps of the ACT-bound attention inner loop instead of between
            # cores (the PE executes its stream in order).
            from collections import deque

            fillers = deque()

            def pump(n):
                for _ in range(n):
                    while fillers:
                        try:
                            next(fillers[0])
                            break
                        except StopIteration:
                            fillers.popleft()
                    else:
                        return

            def attn_core(pair, cq, per_kt=2):
                """Heads 2*pair, 2*pair+1 for query chunk cq; the two heads'
                score matmuls run concurrently on PE row-groups 0-63/64-127.
                Returns the two accumulation psum tiles (rows 0..63 =
                sum(exp*v), row 64 = sum(exp))."""
                ft = pair
                csl = slice(cq * QC, (cq + 1) * QC)
                ps_o = [
                    oaccpool.tile([D + 1, QC], F32, name=f"ps_o{s}", tag="oacc")
                    for s in range(2)
                ]
                for kt in range(NST):
                    et_t = epool.tile([P, 2, QC], F32R, name="et_t", tag="et_t")
                    ps_s = pspool.tile([P, 2, QC], F32, name="ps_s", tag="ps_s")
                    for sub in range(2):
                        lo, hi = sub * D, (sub + 1) * D
                        nc.tensor.matmul(
                            ps_s[:, sub, :],
                            kts[ft][lo:hi, kt * P : (kt + 1) * P],
                            qts[ft][lo:hi, csl],
                            start=True,
                            stop=True,
                        )
                    nc.scalar.activation(
                        out=et_t,
                        in_=ps_s,
                        func=mybir.ActivationFunctionType.Exp,
                        scale=SM,
                    )
                    for sub in range(2):
                        nc.tensor.matmul(
                            ps_o[sub],
                            v_tiles[kt][:, 2 * pair + sub, :],
                            et_t[:, sub, :],
                            start=(kt == 0),
                            stop=(kt == NST - 1),
                        )
                    if kt > 0:
                        pump(per_kt)
                return ps_o

            def attn_drain(ps_o, tail=False):
                """Copy both accumulators (incl. the sum row) to SBUF right
                away so the psum slots free early. In the tail the two copies
                run on different engines (ACT finished its last exp, DVE is
                about to be busy with normalizes) so they overlap."""
                o_full = []
                for sub in range(2):
                    of = epool.tile([D + 1, QC], F32, name="o_hat", tag="o_hat", bufs=4)
                    if tail and sub == 0:
                        nc.scalar.copy(of, ps_o[sub])
                    else:
                        nc.vector.tensor_copy(of, ps_o[sub])
                    o_full.append(of)
                return o_full

            def bcast_recip(o_full):
                """Reciprocal of each sum row, partition-broadcast on the
                (otherwise idle) GPSIMD engine. No PE/ACT work."""
                bcs = []
                for sub in range(2):
                    rec = spool.tile([1, QC], F32, name="rec", tag="rec", bufs=1)
                    nc.vector.reciprocal(rec, o_full[sub][D : D + 1, :])
                    bc = spool.tile([D, QC], F32, name="bc", tag="bc", bufs=4)
                    nc.gpsimd.partition_broadcast(bc, rec)
                    bcs.append(bc)
                return bcs

            def attn_finish(pair, cq, o_full):
                """Normalize a pair-0 chunk (full-width multiply)."""
                csl = slice(cq * QC, (cq + 1) * QC)
                bcs = bcast_recip(o_full)
                for sub in range(2):
                    lo, hi = sub * D, (sub + 1) * D
                    nc.vector.tensor_mul(
                        ots[pair][lo:hi, csl], o_full[sub][0:D, :], bcs[sub]
                    )

            def finish_outproj_units(cq, o_full, bcs, tail=False):
                """Pair-1 normalize pipelined with the output projection at
                s-tile granularity (shortens the kernel tail). In the tail
                the PSUM->SBUF copies ride the idle ACT engine instead of
                DVE."""
                for sti in range(NQ):
                    st = cq * NQ + sti
                    ssl = slice(sti * P, (sti + 1) * P)
                    for sub in range(2):
                        lo, hi = sub * D, (sub + 1) * D
                        if tail:
                            # tail bcs live in PSUM (PE broadcast); GPSIMD
                            # cannot read PSUM, so stay on DVE.
                            nc.vector.tensor_mul(
                                ots[1][lo:hi, st * P : (st + 1) * P],
                                o_full[sub][0:D, ssl],
                                bcs[sub][:, ssl],
                            )
                        else:
                            # all-SBUF multiply: offload to the idle
                            # Pool/GPSIMD engine, freeing DVE for the
                            # PSUM->SBUF copies it alone (with ACT) can do.
                            nc.gpsimd.tensor_mul(
                                ots[1][lo:hi, st * P : (st + 1) * P],
                                o_full[sub][0:D, ssl],
                                bcs[sub][:, ssl],
                            )
                    yield
                    out_sb = outpool.tile([P, E], F32, name="out_sb", tag="out_sb")
                    for gc in range(2):
                        ps_out = popool.tile([P, QC], F32, name="ps_out", tag="po")
                        for ft in range(FT):
                            nc.tensor.matmul(
                                ps_out,
                                ots[ft][:, st * P : (st + 1) * P],
                                wo[:, ft, gc * QC : (gc + 1) * QC],
                                start=(ft == 0),
                                stop=(ft == FT - 1),
                            )
                            yield
                        osl = slice(gc * QC, (gc + 1) * QC)
                        if tail:
                            # alternate engines so the two copies overlap,
                            # and DMA each half as soon as it lands -- the
                            # last tile's DMA latency is the kernel tail.
                            if gc == 0:
                                nc.scalar.activation(
                                    out=out_sb[:, osl],
                                    in_=ps_out,
                                    func=mybir.ActivationFunctionType.Copy,
                                )
                            else:
                                nc.vector.tensor_copy(out_sb[:, osl], ps_out)
                            nc.sync.dma_start(
                                out=out_d.ap()[st * P : (st + 1) * P, osl],
                                in_=out_sb[:, osl],
                            )
                        else:
                            nc.vector.tensor_copy(out_sb[:, osl], ps_out)
                        yield
                    if not tail:
                        nc.sync.dma_start(
                            out=out_d.ap()[st * P : (st + 1) * P, :], in_=out_sb
                        )

            # Emission order = scheduler priority. Attention cores are
            # emitted right after the projections of their own chunk, so the
            # first exp fires as soon as chunk-0 data exists; later-chunk
            # projections backfill PE whenever attention is dep-blocked.
            def proj1_units():
                for cq in range(NQ):
                    for w_tile, dst, which in ((wk, kts, "k1"), (wq, qts, "q1")):
                        ps = popool.tile([P, QC], F32, name=f"ps_{which}", tag="po")
                        for et in range(NE):
                            nc.tensor.matmul(
                                ps,
                                w_tile[:, et, P : 2 * P],
                                xts[et][:, cq * QC : (cq + 1) * QC],
                                start=(et == 0),
                                stop=(et == NE - 1),
                            )
                            yield
                        nc.vector.tensor_copy(
                            dst[1][:, cq * QC : (cq + 1) * QC], ps
                        )
                        yield

            for cq in range(NQ):
                proj_T(wk, kts, "k0", 0, cq)
                proj_T(wq, qts, "q0", 0, cq)
                for st in range(cq * NQ, (cq + 1) * NQ):
                    proj_v(st)

            PER_KT = {(0, 1): 2}
            for pair in range(2):
                for cq in range(NQ):
                    tail = pair == 1 and cq == NQ - 1
                    ps_o = attn_core(pair, cq, per_kt=PER_KT.get((pair, cq), 2 if pair else 1))
                    of = attn_drain(ps_o, tail=tail)
                    if pair == 0:
                        attn_finish(pair, cq, of)
                    elif not tail:
                        bcs = bcast_recip(of)
                        fillers.append(finish_outproj_units(cq, of, bcs))
                    else:
                        # tail chunk: broadcast via a PE matmul (shortest
                        # latency chain right after the last core); the
                        # reciprocal writes fp32r directly so the PE can
                        # consume it without an extra rounding copy.
                        bcs = []
                        for sub in range(2):
                            rec_r = spool.tile([1, QC], F32R, name="rec_r", tag="rec_r", bufs=1)
                            with nc.allow_low_precision(
                                reason="softmax denom reciprocal rounds to fp32r for the PE broadcast"
                            ):
                                nc.vector.reciprocal(rec_r, of[sub][D : D + 1, :])
                            # oacc slots free right after this chunk's drain;
                            # popool is still churning through the previous
                            # chunk's out-proj copies at this point.
                            ps_bc = oaccpool.tile([D, QC], F32, name="ps_bc", tag="oacc")
                            nc.tensor.matmul(ps_bc, ones, rec_r, start=True, stop=True)
                            bcs.append(ps_bc)
                        fillers.appendleft(
                            finish_outproj_units(cq, of, bcs, tail=True)
                        )
                    if pair == 0 and cq == 0:
                        fillers.append(proj1_units())
            # drain remaining fillers (the last chunk's output projection)
            while fillers:
                pump(64)

    nc.compile()
    return nc


_NC_CACHE = None


def _get_nc():
    global _NC_CACHE
    if _NC_CACHE is None:
        _NC_CACHE = _build()
    return _NC_CACHE


def make_in_maps(x, Wq, Wk, Wv, Wo):
    import ml_dtypes

    in_maps = []
    xTs = [
        np.ascontiguousarray(x[b].T).astype(ml_dtypes.bfloat16) for b in range(B)
    ]
    for c in range(NCORES):
        b, hg = c // GPB, c % GPB
        fsl = slice(hg * FPC, (hg + 1) * FPC)
        in_maps.append({
            "xT": xTs[b],
            "wqT": np.ascontiguousarray(Wq[fsl, :].T).astype(ml_dtypes.bfloat16),
            "wkT": np.ascontiguousarray(Wk[fsl, :].T).astype(ml_dtypes.bfloat16),
            "wvT": np.ascontiguousarray(Wv[fsl, :].T).astype(ml_dtypes.bfloat16),
            "woT": _round_fp32r(Wo[:, fsl].T),
            "ones_lhs": np.ones((1, D), dtype=np.float32),
            "ones_col": np.ones((P, HPC, 1), dtype=np.float32),
        })
    return in_maps


def kernel(x, Wq, bq, Wk, bk, Wv, bv, Wo, bo):
    x = np.asarray(x, dtype=np.float32)
    Wq, Wk, Wv, Wo = (np.asarray(a, dtype=np.float32) for a in (Wq, Wk, Wv, Wo))
    bq, bk, bv, bo = (np.asarray(a, dtype=np.float32) for a in (bq, bk, bv, bo))
    if np.any(bq) or np.any(bk) or np.any(bv):
        # fall back: fold nonzero projection biases into an augmented input
        # row is not implemented; biases are zero for this problem spec.
        raise NotImplementedError("nonzero projection biases not supported")

    nc = _get_nc()
    in_maps = make_in_maps(x, Wq, Wk, Wv, Wo)
    res = run_bass_kernel_spmd(nc, in_maps, core_ids=list(range(NCORES)))
    out = np.empty((B, S, E), dtype=np.float32)
    for b in range(B):
        acc = res.results[b * GPB]["out"].astype(np.float32).copy()
        for hg in range(1, GPB):
            acc += res.results[b * GPB + hg]["out"]
        out[b] = acc
    out += bo[None, None, :]
    return out



# revision 24
# speedup vs baseline: 1.0467x; 1.0242x over previous
"""Trainium2 Bass kernel: multi-head attention (B=2, S=2048, E=1024, H=16).

Sharding: 8 cores = 2 batches x 4 head-groups. Core c handles batch c//4 and
heads [4*(c%4), 4*(c%4)+4) (256 feature columns of the projections).

Per-core device program (all matmuls in fp32r):
  - inputs: xT [E,S] (host-transposed x[b]), wqT/wkT/wvT [E,256] (host-
    transposed row-slices of Wq/Wk/Wv), woT [256,E] (host-transposed column
    slice of Wo).
  - qT,kT [256,S] = (x @ W^T)^T per head-group, computed directly in [f,s]
    layout; v [S,256] in [s,f] layout with a ones column appended per head.
  - per (head, qi-chunk): scores^T tiles [128 kj, 512 qi] on PE, exp on ACT
    (sm_scale folded into the activation scale), attn@v accumulated on PE with
    the ones column producing the softmax denominator in partition 64,
    then reciprocal + GPSIMD partition-broadcast + multiply to normalize;
    output kept in [f, s] layout for the output projection.
  - out_partial [S,E] = o^T^T @ Wo^T column-slice; host sums 4 partials per
    batch and adds bo.
"""

import numpy as np

import concourse.tile as tile
import concourse.mybir as mybir
from concourse import bacc
from concourse.bass_utils import run_bass_kernel_spmd

B, S, E, H, D = 2, 2048, 1024, 16, 64
NCORES = 8
GPB = NCORES // B      # head-groups (cores) per batch = 4
HPC = H // GPB         # heads per core = 4
FPC = HPC * D          # feature cols per core = 256
SM = float(D) ** -0.5  # softmax scale

F32 = mybir.dt.float32
F32R = mybir.dt.float32r
BF16 = mybir.dt.bfloat16

P = 128
NE = E // P            # 8 e-tiles
NST = S // P           # 16 s-tiles (key tiles)
NQ = 4                 # qi chunks
QC = S // NQ           # 512
KTG = 2                # k-tiles per psum/exp group
NKG = NST // KTG       # 8 groups
FT = FPC // P          # 2 f-tiles per core


def _round_fp32r(a: np.ndarray) -> np.ndarray:
    """Round fp32 to the fp32r encoding (RNE to 12-bit mantissa)."""
    u = np.ascontiguousarray(a, dtype=np.float32).view(np.uint32)
    lo = u & np.uint32(0xFFF)
    base = u & ~np.uint32(0xFFF)
    rup = (lo > 0x800) | ((lo == 0x800) & (((base >> np.uint32(12)) & np.uint32(1)) == 1))
    out = base + (rup.astype(np.uint32) << np.uint32(12))
    return out.view(np.float32)


def _build():
    nc = bacc.Bacc("TRN2", target_bir_lowering=False, debug=False)

    xT_d = nc.dram_tensor("xT", [E, S], BF16, kind="ExternalInput")
    wq_d = nc.dram_tensor("wqT", [E, FPC], BF16, kind="ExternalInput")
    wk_d = nc.dram_tensor("wkT", [E, FPC], BF16, kind="ExternalInput")
    wv_d = nc.dram_tensor("wvT", [E, FPC], BF16, kind="ExternalInput")
    wo_d = nc.dram_tensor("woT", [FPC, E], F32R, kind="ExternalInput")
    ones_lhs_d = nc.dram_tensor("ones_lhs", [1, D], F32R, kind="ExternalInput")
    ones_col_d = nc.dram_tensor("ones_col", [P, HPC, 1], F32R, kind="ExternalInput")
    out_d = nc.dram_tensor("out", [S, E], BF16, kind="ExternalOutput")

    with tile.TileContext(nc) as tc:
        with (
            tc.tile_pool(name="wpool", bufs=1) as wpool,
            tc.tile_pool(name="xpool", bufs=1) as xpool,
            tc.tile_pool(name="qkpool", bufs=1) as qkpool,
            tc.tile_pool(name="vpool", bufs=1) as vpool,
            tc.tile_pool(name="opool", bufs=1) as opool,
            tc.tile_pool(name="epool", bufs=3) as epool,
            tc.tile_pool(name="spool", bufs=2) as spool,
            tc.tile_pool(name="outpool", bufs=3) as outpool,
            tc.tile_pool(name="pspool", bufs=2, space="PSUM") as pspool,
            tc.tile_pool(name="popool", bufs=2, space="PSUM") as popool,
            tc.tile_pool(name="oaccpool", bufs=2, space="PSUM") as oaccpool,
        ):
            # ---- weights / constants -------------------------------------
            wq = wpool.tile([P, NE, FPC], BF16, name="wq")
            wk = wpool.tile([P, NE, FPC], BF16, name="wk")
            wv = wpool.tile([P, NE, FPC], BF16, name="wv")
            wo = wpool.tile([P, FT, E], F32R, name="wo")
            ones = wpool.tile([1, D], F32R, name="ones")

            wk_r = wk_d.ap().rearrange("(t p) f -> p t f", p=P)
            wq_r = wq_d.ap().rearrange("(t p) f -> p t f", p=P)
            wv_r = wv_d.ap().rearrange("(t p) f -> p t f", p=P)

            # ---- x^T (chunk-major DMA so compute starts early) -----------
            xT_r = xT_d.ap().rearrange("(t p) s -> p t s", p=P)
            # x in bf16: halves the startup-critical HBM traffic; the ~0.2%
            # relative quantization is well inside the 2e-2 tolerance. One
            # [P, NE, S] tile so a whole query-chunk's worth of x moves in a
            # single dma_start -- each dma_start costs ~625ns of serialized
            # HWDGE + 650ns DGE + 900ns sem-prop, so instruction count, not
            # bytes, dominated the old per-et startup stream.
            xtile = xpool.tile([P, NE, S], BF16, name="xtile")
            xts = [xtile[:, et, :] for et in range(NE)]
            # Chunk 0 is latency-critical: stage it so the k0 accumulation
            # (et order) starts after ~600KB instead of the full 2MB.
            nc.sync.dma_start(out=wk[:, 0:1, 0:P], in_=wk_r[:, 0:1, 0:P])
            nc.sync.dma_start(out=xtile[:, 0:1, 0:QC], in_=xT_r[:, 0:1, 0:QC])
            nc.sync.dma_start(out=wk[:, 1:NE, 0:P], in_=wk_r[:, 1:NE, 0:P])
            nc.sync.dma_start(out=xtile[:, 1:4, 0:QC], in_=xT_r[:, 1:4, 0:QC])
            nc.sync.dma_start(out=xtile[:, 4:NE, 0:QC], in_=xT_r[:, 4:NE, 0:QC])
            nc.sync.dma_start(out=wq[:, :, 0:P], in_=wq_r[:, :, 0:P])
            nc.sync.dma_start(out=wv[:, 0:4, :], in_=wv_r[:, 0:4, :])
            nc.sync.dma_start(out=wv[:, 4:NE, :], in_=wv_r[:, 4:NE, :])
            nc.sync.dma_start(out=ones, in_=ones_lhs_d.ap())
            for cq in range(1, NQ):
                csl = slice(cq * QC, (cq + 1) * QC)
                nc.sync.dma_start(out=xtile[:, :, csl], in_=xT_r[:, :, csl])
                if cq == 2:
                    nc.sync.dma_start(out=wk[:, :, P:FPC], in_=wk_r[:, :, P:FPC])
                    nc.sync.dma_start(out=wq[:, :, P:FPC], in_=wq_r[:, :, P:FPC])

            nc.sync.dma_start(out=wo, in_=wo_d.ap().rearrange("(t p) g -> p t g", p=P))

            # ---- v projection: v[s, f] with ones col per head ------------
            v_tiles = [
                vpool.tile([P, HPC, D + 1], F32R, name=f"v{st}", tag=f"v{st}")
                for st in range(NST)
            ]

            def proj_v(st):
                vt = v_tiles[st]
                nc.sync.dma_start(out=vt[:, :, D : D + 1], in_=ones_col_d.ap())
                ps_v = popool.tile([P, FPC], F32, name="ps_v", tag="po")
                for et in range(NE):
                    nc.tensor.matmul(
                        ps_v,
                        xts[et][:, st * P : (st + 1) * P],
                        wv[:, et, :],
                        start=(et == 0),
                        stop=(et == NE - 1),
                    )
                nc.vector.tensor_copy(
                    vt[:, :, 0:D], ps_v.rearrange("p (h d) -> p h d", d=D)
                )

            # ---- q^T / k^T projections: [f, s] ---------------------------
            def proj_T(w_tile, dst_tiles, which, ft, cq):
                ps = popool.tile([P, QC], F32, name=f"ps_{which}", tag="po")
                for et in range(NE):
                    nc.tensor.matmul(
                        ps,
                        w_tile[:, et, ft * P : (ft + 1) * P],
                        xts[et][:, cq * QC : (cq + 1) * QC],
                        start=(et == 0),
                        stop=(et == NE - 1),
                    )
                nc.vector.tensor_copy(
                    dst_tiles[ft][:, cq * QC : (cq + 1) * QC], ps
                )

            kts = [qkpool.tile([P, S], F32R, name=f"kt{ft}", tag=f"kt{ft}") for ft in range(FT)]
            qts = [qkpool.tile([P, S], F32R, name=f"qt{ft}", tag=f"qt{ft}") for ft in range(FT)]
            ots = [opool.tile([P, S], F32R, name=f"ot{ft}", tag=f"ot{ft}") for ft in range(FT)]

            # Filler machinery: generators that emit one PE-side instruction
            # per next() call. attn_core drains a couple of units after each
            # kt step, so independent matmul work lands inside the PE idle
            # gaps of the ACT-bound attention inner loop instead of between
            # cores (the PE executes its stream in order).
            from collections import deque

            fillers = deque()

            def pump(n):
                for _ in range(n):
                    while fillers:
                        try:
                            next(fillers[0])
                            break
                        except StopIteration:
                            fillers.popleft()
                    else:
                        return

            def attn_core(pair, cq, per_kt=2):
                """Heads 2*pair, 2*pair+1 for query chunk cq; the two heads'
                score matmuls run concurrently on PE row-groups 0-63/64-127.
                Returns the two accumulation psum tiles (rows 0..63 =
                sum(exp*v), row 64 = sum(exp))."""
                ft = pair
                csl = slice(cq * QC, (cq + 1) * QC)
                ps_o = [
                    oaccpool.tile([D + 1, QC], F32, name=f"ps_o{s}", tag="oacc")
                    for s in range(2)
                ]
                for kt in range(NST):
                    et_t = epool.tile([P, 2, QC], F32R, name="et_t", tag="et_t")
                    ps_s = pspool.tile([P, 2, QC], F32, name="ps_s", tag="ps_s")
                    for sub in range(2):
                        lo, hi = sub * D, (sub + 1) * D
                        nc.tensor.matmul(
                            ps_s[:, sub, :],
                            kts[ft][lo:hi, kt * P : (kt + 1) * P],
                            qts[ft][lo:hi, csl],
                            start=True,
                            stop=True,
                        )
                    nc.scalar.activation(
                        out=et_t,
                        in_=ps_s,
                        func=mybir.ActivationFunctionType.Exp,
                        scale=SM,
                    )
                    for sub in range(2):
                        nc.tensor.matmul(
                            ps_o[sub],
                            v_tiles[kt][:, 2 * pair + sub, :],
                            et_t[:, sub, :],
                            start=(kt == 0),
                            stop=(kt == NST - 1),
                        )
                    if kt > 0:
                        pump(per_kt)
                return ps_o

            def attn_drain(ps_o, tail=False):
                """Copy both accumulators (incl. the sum row) to SBUF right
                away so the psum slots free early. In the tail the two copies
                run on different engines (ACT finished its last exp, DVE is
                about to be busy with normalizes) so they overlap."""
                o_full = []
                for sub in range(2):
                    of = epool.tile([D + 1, QC], F32, name="o_hat", tag="o_hat", bufs=4)
                    if tail and sub == 0:
                        nc.scalar.copy(of, ps_o[sub])
                    else:
                        nc.vector.tensor_copy(of, ps_o[sub])
                    o_full.append(of)
                return o_full

            def bcast_recip(o_full):
                """Reciprocal of each sum row, partition-broadcast on the
                (otherwise idle) GPSIMD engine. No PE/ACT work."""
                bcs = []
                for sub in range(2):
                    rec = spool.tile([1, QC], F32, name="rec", tag="rec", bufs=1)
                    nc.vector.reciprocal(rec, o_full[sub][D : D + 1, :])
                    bc = spool.tile([D, QC], F32, name="bc", tag="bc", bufs=4)
                    nc.gpsimd.partition_broadcast(bc, rec)
                    bcs.append(bc)
                return bcs

            def attn_finish(pair, cq, o_full):
                """Normalize a pair-0 chunk (full-width multiply)."""
                csl = slice(cq * QC, (cq + 1) * QC)
                bcs = bcast_recip(o_full)
                for sub in range(2):
                    lo, hi = sub * D, (sub + 1) * D
                    nc.vector.tensor_mul(
                        ots[pair][lo:hi, csl], o_full[sub][0:D, :], bcs[sub]
                    )

            def finish_outproj_units(cq, o_full, bcs, tail=False):
                """Pair-1 normalize pipelined with the output projection at
                s-tile granularity (shortens the kernel tail). In the tail
                the PSUM->SBUF copies ride the idle ACT engine instead of
                DVE."""
                # Normalize all four s-tiles first so the muls get a pump
                # head-start over their out-proj matmuls; alternate DVE and
                # the otherwise-idle Pool engine (Pool cannot read PSUM, so
                # the tail -- whose bcs live in PSUM -- stays on DVE).
                for sti in range(NQ):
                    st = cq * NQ + sti
                    ssl = slice(sti * P, (sti + 1) * P)
                    for sub in range(2):
                        lo, hi = sub * D, (sub + 1) * D
                        eng = (
                            nc.vector
                            if tail or (sti + sub) % 2 == 0
                            else nc.gpsimd
                        )
                        eng.tensor_mul(
                            ots[1][lo:hi, st * P : (st + 1) * P],
                            o_full[sub][0:D, ssl],
                            bcs[sub][:, ssl],
                        )
                    yield
                for sti in range(NQ):
                    st = cq * NQ + sti
                    out_sb = outpool.tile([P, E], BF16, name="out_sb", tag="out_sb")
                    for gc in range(2):
                        ps_out = popool.tile([P, QC], F32, name="ps_out", tag="po")
                        for ft in range(FT):
                            nc.tensor.matmul(
                                ps_out,
                                ots[ft][:, st * P : (st + 1) * P],
                                wo[:, ft, gc * QC : (gc + 1) * QC],
                                start=(ft == 0),
                                stop=(ft == FT - 1),
                            )
                            yield
                        osl = slice(gc * QC, (gc + 1) * QC)
                        if tail:
                            # alternate engines so the two copies overlap,
                            # and DMA each half as soon as it lands -- the
                            # last tile's DMA latency is the kernel tail.
                            if gc == 0:
                                nc.scalar.activation(
                                    out=out_sb[:, osl],
                                    in_=ps_out,
                                    func=mybir.ActivationFunctionType.Copy,
                                )
                            else:
                                nc.vector.tensor_copy(out_sb[:, osl], ps_out)
                            nc.sync.dma_start(
                                out=out_d.ap()[st * P : (st + 1) * P, osl],
                                in_=out_sb[:, osl],
                            )
                        else:
                            nc.vector.tensor_copy(out_sb[:, osl], ps_out)
                        yield
                    if not tail:
                        nc.sync.dma_start(
                            out=out_d.ap()[st * P : (st + 1) * P, :], in_=out_sb
                        )

            # Emission order = scheduler priority. Attention cores are
            # emitted right after the projections of their own chunk, so the
            # first exp fires as soon as chunk-0 data exists; later-chunk
            # projections backfill PE whenever attention is dep-blocked.
            def proj1_units():
                for cq in range(NQ):
                    for w_tile, dst, which in ((wk, kts, "k1"), (wq, qts, "q1")):
                        ps = popool.tile([P, QC], F32, name=f"ps_{which}", tag="po")
                        for et in range(NE):
                            nc.tensor.matmul(
                                ps,
                                w_tile[:, et, P : 2 * P],
                                xts[et][:, cq * QC : (cq + 1) * QC],
                                start=(et == 0),
                                stop=(et == NE - 1),
                            )
                            yield
                        nc.vector.tensor_copy(
                            dst[1][:, cq * QC : (cq + 1) * QC], ps
                        )
                        yield

            for cq in range(NQ):
                proj_T(wk, kts, "k0", 0, cq)
                proj_T(wq, qts, "q0", 0, cq)
                for st in range(cq * NQ, (cq + 1) * NQ):
                    proj_v(st)

            PER_KT = {(0, 1): 2}
            for pair in range(2):
                for cq in range(NQ):
                    tail = pair == 1 and cq == NQ - 1
                    ps_o = attn_core(pair, cq, per_kt=PER_KT.get((pair, cq), 2 if pair else 1))
                    of = attn_drain(ps_o, tail=tail)
                    if pair == 0:
                        attn_finish(pair, cq, of)
                    elif not tail:
                        bcs = bcast_recip(of)
                        fillers.append(finish_outproj_units(cq, of, bcs))
                    else:
                        # tail chunk: broadcast via a PE matmul (shortest
                        # latency chain right after the last core); the
                        # reciprocal writes fp32r directly so the PE can
                        # consume it without an extra rounding copy.
                        bcs = []
                        for sub in range(2):
                            rec_r = spool.tile([1, QC], F32R, name="rec_r", tag="rec_r", bufs=1)
                            with nc.allow_low_precision(
                                reason="softmax denom reciprocal rounds to fp32r for the PE broadcast"
                            ):
                                nc.vector.reciprocal(rec_r, of[sub][D : D + 1, :])
                            # oacc slots free right after this chunk's drain;
                            # popool is still churning through the previous
                            # chunk's out-proj copies at this point.
                            ps_bc = oaccpool.tile([D, QC], F32, name="ps_bc", tag="oacc")
                            nc.tensor.matmul(ps_bc, ones, rec_r, start=True, stop=True)
                            bcs.append(ps_bc)
                        fillers.appendleft(
                            finish_outproj_units(cq, of, bcs, tail=True)
                        )
                    if pair == 0 and cq == 0:
                        fillers.append(proj1_units())
            # drain remaining fillers (the last chunk's output projection)
            while fillers:
                pump(64)

    nc.compile()
    return nc


_NC_CACHE = None


def _get_nc():
    global _NC_CACHE
    if _NC_CACHE is None:
        _NC_CACHE = _build()
    return _NC_CACHE


def make_in_maps(x, Wq, Wk, Wv, Wo):
    import ml_dtypes

    in_maps = []
    xTs = [
        np.ascontiguousarray(x[b].T).astype(ml_dtypes.bfloat16) for b in range(B)
    ]
    for c in range(NCORES):
        b, hg = c // GPB, c % GPB
        fsl = slice(hg * FPC, (hg + 1) * FPC)
        in_maps.append({
            "xT": xTs[b],
            "wqT": np.ascontiguousarray(Wq[fsl, :].T).astype(ml_dtypes.bfloat16),
            "wkT": np.ascontiguousarray(Wk[fsl, :].T).astype(ml_dtypes.bfloat16),
            "wvT": np.ascontiguousarray(Wv[fsl, :].T).astype(ml_dtypes.bfloat16),
            "woT": _round_fp32r(Wo[:, fsl].T),
            "ones_lhs": np.ones((1, D), dtype=np.float32),
            "ones_col": np.ones((P, HPC, 1), dtype=np.float32),
        })
    return in_maps


def kernel(x, Wq, bq, Wk, bk, Wv, bv, Wo, bo):
    x = np.asarray(x, dtype=np.float32)
    Wq, Wk, Wv, Wo = (np.asarray(a, dtype=np.float32) for a in (Wq, Wk, Wv, Wo))
    bq, bk, bv, bo = (np.asarray(a, dtype=np.float32) for a in (bq, bk, bv, bo))
    if np.any(bq) or np.any(bk) or np.any(bv):
        # fall back: fold nonzero projection biases into an augmented input
        # row is not implemented; biases are zero for this problem spec.
        raise NotImplementedError("nonzero projection biases not supported")

    nc = _get_nc()
    in_maps = make_in_maps(x, Wq, Wk, Wv, Wo)
    res = run_bass_kernel_spmd(nc, in_maps, core_ids=list(range(NCORES)))
    out = np.empty((B, S, E), dtype=np.float32)
    for b in range(B):
        acc = res.results[b * GPB]["out"].astype(np.float32)
        for hg in range(1, GPB):
            acc = acc + res.results[b * GPB + hg]["out"].astype(np.float32)
        out[b] = acc
    out += bo[None, None, :]
    return out

